# revision 10
# baseline (speedup 1.0000x reference)
"""Distributed Trainium2 Bass kernel for a dense pre-LN transformer block.

Problem: x:[4,2048,1024] f32; per-head QKV (H=16, HS=64), causal attention,
out-proj + residual, pre-LN MLP (4x) + residual.

Sharding over 8 NeuronCores ("batch-pair" layout):
- Core pair (2b, 2b+1) owns batch b.  Within the batch, the 16 token chunks
  (128 tokens each) are split between the two cores in the balanced
  interleave {0,3,4,7,8,11,12,15} / {1,2,5,6,9,10,13,14}.
- Every core computes LN1 / QKV / all 16 heads of attention for its OWN
  1024 (packed) query tokens, then out-proj / LN2 / MLP for the same
  tokens.  The ONLY communication is a 1 MB fp8 AllGather of the
  transposed LN1 output between the two partners (pair replica groups, in
  two token halves); K and V for the full batch are computed locally from
  the gathered h1.
- SPMD uniformity: one graph for all cores.  Key blocks are indexed by
  (AllGather slice, packed position); the query suffix for key block k
  always starts at packed column (k % NTT)*128; all per-core causal
  asymmetry (tri vs full vs zero on the first suffix block) lives in a
  host-supplied mask tensor `msk`.

Scheduling: attention runs in token halves.  Half A needs only the
first-half key blocks, so it starts right after the first AllGather; the
second-half K/V matmuls are interleaved as PE filler under half A's
exp-bound window.  Under half B's exp window the PE runs out-proj, LN2,
and MLP1 for half A's tokens.  MLP1/MLP2 for half B close out the rep.

Precision: h1/Q/K/V/scores/AV in fp8e4 with f32 PSUM (QKV matmuls use
DoubleRow for 2x PE rate); out-proj/MLP in bf16; LN, softmax normalize,
PSUM accumulation in f32; residual stream x2 in bf16.  Softmax skips the
max subtraction (scores are O(1)) and takes its denominator from a ones
column appended to V.
"""

import numpy as np
import ml_dtypes

import concourse.bass as bass
import concourse.bacc as bacc
import concourse.tile as tile
import concourse.mybir as mybir
from concourse.bass_utils import run_bass_kernel_spmd
from concourse.masks import make_identity

BF16 = mybir.dt.bfloat16
F32 = mybir.dt.float32
F8 = mybir.dt.float8e4
NP_BF16 = ml_dtypes.bfloat16
NP_F8 = ml_dtypes.float8_e4m3
P = 128
EPS = 1e-5


def own_chunks(nch: int, parity: int):
    """Balanced chunk interleave: groups of 4 -> even core {0,3}, odd {1,2}."""
    out = []
    for g in range(nch // 4):
        out += ([4 * g, 4 * g + 3] if parity == 0 else [4 * g + 1, 4 * g + 2])
    return out


class Cfg:
    def __init__(self, B=4, T=2048, D=1024, DH=4096, HS=64, NC=8):
        self.B, self.T, self.D, self.DH, self.HS, self.NC = B, T, D, DH, HS, NC
        self.H = D // HS                  # 16 heads
        self.HP = self.H // 2             # head pairs (two heads per 128 rows)
        self.NCH = T // P                 # 128-token chunks per batch (16)
        self.NTT = self.NCH // 2          # packed chunks per core (8)
        self.TSH = self.NTT * P           # own tokens per core (1024)
        self.TSH2 = self.TSH // 2
        self.NKB = 2 * self.NTT           # key blocks (16)
        self.NPR = self.NKB // 2          # DoubleRow key-block pairs (8)
        self.DC = D // P                  # dim chunks (8)
        self.HC = DH // P                 # hidden chunks (32)
        assert self.NCH % 4 == 0 and D % P == 0 and DH % P == 0
        assert HS * 2 == P


FULL = Cfg()
SMALL = Cfg(B=4, T=512, D=1024, DH=1024)


def build_nc(cfg: Cfg, reps: int = 1):
    nc = bacc.Bacc("TRN2", target_bir_lowering=False, debug=False,
                   num_devices=cfg.NC)
    B, T, D, DH, HS, NC = cfg.B, cfg.T, cfg.D, cfg.DH, cfg.HS, cfg.NC
    TSH, TSH2, NTT, NKB, NPR, DC, HC, HP = (
        cfg.TSH, cfg.TSH2, cfg.NTT, cfg.NKB, cfg.NPR, cfg.DC, cfg.HC, cfg.HP)
    H = cfg.H
    NBH = NTT // 2                        # key blocks per (slice, half)
    TH = TSH // 2                         # tokens per attention half
    NTT2 = NTT // 2                       # packed chunks per half
    rg = [[2 * g, 2 * g + 1] for g in range(NC // 2)]   # pair replica groups

    def t0_of(m):
        return (2 * m % NTT) * P          # pair anchor column

    def segs(lo, hi, w=512):
        return [(s, min(hi, s + w)) for s in range(lo, hi, w)]

    # ---- parameters (per-core shards supplied host-side) ----
    x_ext = nc.declare_dram_parameter("x", [TSH, D], F32, isOutput=False)
    xr_ext = nc.declare_dram_parameter("xr", [TSH, D], BF16, isOutput=False)
    wq_ext = nc.declare_dram_parameter("wq", [D, D], F8, isOutput=False)
    wk_ext = nc.declare_dram_parameter("wk", [D, D], F8, isOutput=False)
    wv_ext = nc.declare_dram_parameter("wv", [D, D], F8, isOutput=False)
    wo_ext = nc.declare_dram_parameter("wo", [D, D], BF16, isOutput=False)
    w1_ext = nc.declare_dram_parameter("w1", [D, DH], BF16, isOutput=False)
    w2_ext = nc.declare_dram_parameter("w2", [DH, D], BF16, isOutput=False)
    g1_ext = nc.declare_dram_parameter("g1", [1, D], F32, isOutput=False)
    g2_ext = nc.declare_dram_parameter("g2", [1, D], F32, isOutput=False)
    b2_ext = nc.declare_dram_parameter("b2", [1, D], F32, isOutput=False)
    b1t_ext = nc.declare_dram_parameter("b1t", [P, HC], F32, isOutput=False)
    msk_ext = nc.declare_dram_parameter("msk", [P, NKB, P], BF16,
                                        isOutput=False)
    out_ext = nc.declare_dram_parameter("out", [TSH, D], F32, isOutput=True)

    # ---- internal DRAM: AllGather bounce buffers, split in token halves ----
    h1t_bounce_a = nc.dram_tensor("h1t_bounce_a", [D, TSH2], F8)
    h1t_bounce_b = nc.dram_tensor("h1t_bounce_b", [D, TSH2], F8)
    h1t_pair_a = nc.dram_tensor("h1t_pair_a", [2 * D, TSH2], F8)
    h1t_pair_b = nc.dram_tensor("h1t_pair_b", [2 * D, TSH2], F8)

    def bcast_row(handle):
        return bass.AP(tensor=handle, offset=0, ap=[[0, P], [1, D]])

    with tile.TileContext(nc) as tc:
        with tc.tile_pool(name="const", bufs=1) as const, \
             tc.tile_pool(name="ln", bufs=2) as ln_pool:
            ident = const.tile([P, P], BF16)
            eps_t = const.tile([P, 1], F32)
            zero_t = const.tile([P, 512], F32)
            g1_sb = const.tile([P, D], F32)
            g2_sb = const.tile([P, D], F32)
            b2_sb = const.tile([P, D], F32)
            b1t_sb = const.tile([P, HC], F32)
            msk_sb = const.tile([P, NKB, P], BF16)

            def layernorm(src_ap, g_sb, dst_bf):
                """LN over free axis D of [P, D] src -> dst tile.

                beta is zero for this problem, so the final scalar-tensor-
                tensor writes dst directly: (x*rstd - mu*rstd) * g."""
                stats = ln_pool.tile([P, D // 512, 6], F32, tag="stats")
                for s in range(D // 512):
                    nc.vector.bn_stats(out=stats[:, s, :],
                                       in_=src_ap[:, s * 512:(s + 1) * 512])
                mv = ln_pool.tile([P, 2], F32, tag="mv")
                nc.vector.bn_aggr(out=mv, in_=stats)
                std = ln_pool.tile([P, 1], F32, tag="std")
                nc.scalar.activation(out=std, in_=mv[:, 1:2],
                                     func=mybir.ActivationFunctionType.Sqrt,
                                     bias=eps_t)
                rstd = ln_pool.tile([P, 1], F32, tag="rstd")
                nc.vector.reciprocal(out=rstd, in_=std)
                mu_rstd = ln_pool.tile([P, 1], F32, tag="murstd")
                nc.vector.tensor_mul(out=mu_rstd, in0=mv[:, 0:1], in1=rstd)
                tmp = ln_pool.tile([P, D], F32, tag="lntmp")
                nc.scalar.activation(out=tmp, in_=src_ap,
                                     func=mybir.ActivationFunctionType.Copy,
                                     scale=rstd)
                nc.vector.scalar_tensor_tensor(
                    out=dst_bf, in0=tmp, scalar=mu_rstd, in1=g_sb,
                    op0=mybir.AluOpType.subtract, op1=mybir.AluOpType.mult)

            for _rep in range(reps):
                with tc.tile_pool(name="resid", bufs=1, side="right") as resid:
                    x2_sb = resid.tile([P, NTT, D], BF16)
                    h2t_sb = resid.tile([P, DC, TSH], BF16)
                    act_a = resid.tile([P, HC, TH], BF16)

                    with tc.tile_pool(name="attp", bufs=1) as attp:
                        att_sb = attp.tile([P, HP, TSH], BF16)

                        with tc.tile_pool(name="qkvsb", bufs=1) as qkvsb:
                            qt_sb = qkvsb.tile([P, HP, TSH], F8)
                            kt_sb = qkvsb.tile([P, HP, NKB * P], F8)
                            vt_sb = qkvsb.tile([P, NKB, H, 72], F8)

                            # === Phase 1: LN1 + transpose + pair-AllGather ==
                            with tc.tile_pool(name="h1tp", bufs=1) as h1tp:
                                h1t_sb = h1tp.tile([P, DC, TSH], F8)
                                with tc.tile_pool(name="xin", bufs=1) as xin, \
                                     tc.tile_pool(name="tr_psum", bufs=2,
                                                  space="PSUM") as trp:
                                    x_tiles = []
                                    for i in range(NTT):
                                        x_t = xin.tile([P, D], F32,
                                                       tag=f"x{i}")
                                        nc.sync.dma_start(
                                            out=x_t,
                                            in_=x_ext[i * P:(i + 1) * P, :])
                                        x_tiles.append(x_t)
                                    nc.sync.dma_start(out=g1_sb,
                                                      in_=bcast_row(g1_ext))
                                    nc.vector.memset(eps_t, EPS)
                                    nc.vector.memset(zero_t, 0.0)
                                    make_identity(nc, ident)
                                    nc.sync.dma_start(out=msk_sb,
                                                      in_=msk_ext[:])
                                    nc.vector.memset(
                                        vt_sb[:, :, :, HS:HS + 1], 1.0)
                                    for i in range(NTT):
                                        h1_bf = ln_pool.tile([P, D], F8,
                                                             tag="h1bf")
                                        layernorm(x_tiles[i], g1_sb, h1_bf)
                                        for dc in range(DC):
                                            pt = trp.tile([P, P], F8)
                                            nc.tensor.transpose(
                                                pt,
                                                h1_bf[:, dc * P:(dc + 1) * P],
                                                ident)
                                            if dc % 2 == 0:
                                                nc.scalar.copy(
                                                    out=h1t_sb[
                                                        :, dc,
                                                        i * P:(i + 1) * P],
                                                    in_=pt)
                                            else:
                                                nc.vector.tensor_copy(
                                                    out=h1t_sb[
                                                        :, dc,
                                                        i * P:(i + 1) * P],
                                                    in_=pt)
                                        if i == NTT // 2 - 1:
                                            nc.sync.dma_start(
                                                out=h1t_bounce_a[:].rearrange(
                                                    "(dc p) t -> p dc t",
                                                    p=P),
                                                in_=h1t_sb[:, :, 0:TSH2])
                                            nc.gpsimd.collective_compute(
                                                "AllGather",
                                                mybir.AluOpType.bypass,
                                                replica_groups=rg,
                                                ins=[h1t_bounce_a[:]],
                                                outs=[h1t_pair_a[:]])
                                        if i == NTT - 1:
                                            nc.sync.dma_start(
                                                out=h1t_bounce_b[:].rearrange(
                                                    "(dc p) t -> p dc t",
                                                    p=P),
                                                in_=h1t_sb[:, :, TSH2:TSH])
                                            nc.gpsimd.collective_compute(
                                                "AllGather",
                                                mybir.AluOpType.bypass,
                                                replica_groups=rg,
                                                ins=[h1t_bounce_b[:]],
                                                outs=[h1t_pair_b[:]])

                                # === Phase 2: Q^T (own tokens, fp8) ===
                                wqv = wq_ext[:].rearrange(
                                    "(dc p) n -> p dc n", p=P)
                                with tc.tile_pool(name="wqs", bufs=2) as wqs, \
                                     tc.tile_pool(name="q_psum", bufs=2,
                                                  space="PSUM") as qp:
                                    for hp in range(HP):
                                        wq_t = wqs.tile([P, DC, P], F8,
                                                        tag="wq")
                                        nc.sync.dma_start(
                                            out=wq_t,
                                            in_=wqv[:, :, hp * P:(hp + 1) * P])
                                        for (s0, s1) in segs(0, TSH):
                                            ps = qp.tile([P, 512], F32,
                                                         tag="qps")
                                            for dc in range(0, DC, 2):
                                                nc.tensor.matmul(
                                                    ps[:, 0:s1 - s0],
                                                    lhsT=wq_t[:, dc:dc + 2,
                                                              :],
                                                    rhs=h1t_sb[:, dc:dc + 2,
                                                               s0:s1],
                                                    start=(dc == 0),
                                                    stop=(dc == DC - 2),
                                                    perf_mode=mybir.
                                                    MatmulPerfMode.DoubleRow)
                                            nc.vector.tensor_copy(
                                                out=qt_sb[:, hp, s0:s1],
                                                in_=ps[:, 0:s1 - s0])
                            # h1t_sb released here

                            # === Phases 3..7 interleaved ===
                            agv_a = h1t_pair_a[:].rearrange(
                                "(s dc p) t -> s p dc t", dc=DC, p=P)
                            agv_b = h1t_pair_b[:].rearrange(
                                "(s dc p) t -> s p dc t", dc=DC, p=P)
                            w1view = w1_ext[:].rearrange(
                                "(dc p) (hc m) -> p dc hc m", p=P, m=P)
                            with tc.tile_pool(name="kvsb", bufs=1) as kvsb, \
                                 tc.tile_pool(name="agp", bufs=2) as agp, \
                                 tc.tile_pool(name="wks", bufs=2) as wks, \
                                 tc.tile_pool(name="wop", bufs=1) as wop, \
                                 tc.tile_pool(name="xrin", bufs=2) as xrin, \
                                 tc.tile_pool(name="w1in", bufs=2) as w1in, \
                                 tc.tile_pool(name="ex", bufs=4) as epool, \
                                 tc.tile_pool(name="dn", bufs=1) as dpool, \
                                 tc.tile_pool(name="k_psum", bufs=1,
                                              space="PSUM") as kp, \
                                 tc.tile_pool(name="v_psum", bufs=1,
                                              space="PSUM") as vp, \
                                 tc.tile_pool(name="sc_psum", bufs=2,
                                              space="PSUM") as scp, \
                                 tc.tile_pool(name="av_psum", bufs=1,
                                              space="PSUM") as avp, \
                                 tc.tile_pool(name="op_psum", bufs=1,
                                              space="PSUM") as opp, \
                                 tc.tile_pool(name="tr2_psum", bufs=1,
                                              space="PSUM") as tr2, \
                                 tc.tile_pool(name="m1_psum", bufs=2,
                                              space="PSUM") as m1p:
                                wv_sb = kvsb.tile([P, DC, D], F8)
                                nc.sync.dma_start(
                                    out=wv_sb,
                                    in_=wv_ext[:].rearrange(
                                        "(dc p) n -> p dc n", p=P))
                                wkv = wk_ext[:].rearrange(
                                    "(dc p) n -> p dc n", p=P)
                                wo_sb = wop.tile([P, DC, D], BF16)
                                nc.sync.dma_start(
                                    out=wo_sb,
                                    in_=wo_ext[:].rearrange(
                                        "(dc p) n -> p dc n", p=P))

                                def kv_units(hf):
                                    """Emission thunks: K and V matmuls for
                                    the hf token half of both AG slices."""
                                    units = []
                                    for s in range(2):
                                        ag_t = agp.tile(
                                            [P, DC, TSH2], F8, tag="ag",
                                            name=f"ag{hf}{s}")
                                        src = (agv_a, agv_b)[hf]

                                        def dma(ag_t=ag_t, s=s):
                                            nc.sync.dma_start(
                                                out=ag_t, in_=src[s])
                                        units.append(dma)
                                        for hp in range(HP):
                                            def kst(ag_t=ag_t, s=s, hp=hp):
                                                wk_t = wks.tile(
                                                    [P, DC, P], F8, tag="wk")
                                                nc.sync.dma_start(
                                                    out=wk_t,
                                                    in_=wkv[:, :, hp * P:
                                                            (hp + 1) * P])
                                                kps = kp.tile([P, TSH2], F32,
                                                              tag="kps")
                                                for dc in range(0, DC, 2):
                                                    nc.tensor.matmul(
                                                        kps,
                                                        lhsT=wk_t[:,
                                                                  dc:dc + 2,
                                                                  :],
                                                        rhs=ag_t[:,
                                                                 dc:dc + 2,
                                                                 :],
                                                        start=(dc == 0),
                                                        stop=(dc == DC - 2),
                                                        perf_mode=mybir.
                                                        MatmulPerfMode.
                                                        DoubleRow)
                                                c0 = (s * NTT
                                                      + hf * NBH) * P
                                                nc.vector.tensor_copy(
                                                    out=kt_sb[:, hp,
                                                              c0:c0 + TSH2],
                                                    in_=kps)
                                            units.append(kst)
                                        for j in range(NBH):
                                            for (f0, f1) in segs(0, D):
                                                def vst(ag_t=ag_t, s=s, j=j,
                                                        f0=f0, f1=f1):
                                                    blk = (s * NTT + hf * NBH
                                                           + j)
                                                    vps = vp.tile(
                                                        [P, 512], F32,
                                                        tag="vps")
                                                    for dc in range(0, DC, 2):
                                                        nc.tensor.matmul(
                                                            vps,
                                                            lhsT=ag_t[
                                                                :, dc:dc + 2,
                                                                j * P:
                                                                (j + 1) * P],
                                                            rhs=wv_sb[
                                                                :, dc:dc + 2,
                                                                f0:f1],
                                                            start=(dc == 0),
                                                            stop=(dc ==
                                                                  DC - 2),
                                                            perf_mode=mybir.
                                                            MatmulPerfMode.
                                                            DoubleRow)
                                                    vv = vps.rearrange(
                                                        "p (h f) -> p h f",
                                                        f=HS)
                                                    nc.vector.tensor_copy(
                                                        out=vt_sb[
                                                            :, blk,
                                                            f0 // HS:
                                                            f1 // HS,
                                                            0:HS],
                                                        in_=vv)
                                                units.append(vst)
                                    return units

                                def attn_head(h, half):
                                    """Scores/exp/AV/normalize for one head
                                    over one token half."""
                                    lo, hi = half * TH, half * TH + TH
                                    hp, ho = h // 2, (h % 2) * HS
                                    pairs = [m for m in range(NPR)
                                             if t0_of(m) < hi]
                                    # last accumulating pair per 256-block
                                    b256 = list(range(lo, hi, 256))
                                    lastw = {b0: max(m for m in pairs
                                                     if max(t0_of(m), lo)
                                                     <= b0)
                                             for b0 in b256}
                                    av = avp.tile([HS + 1, TH], F32,
                                                  tag="av")
                                    for m in pairs:
                                        k0 = 2 * m
                                        t0 = t0_of(m)
                                        r0 = max(t0, lo)
                                        ex2 = epool.tile([P, 2, TH], F8,
                                                         tag="e")
                                        for j in range(2):
                                            k = k0 + j
                                            rj = max(t0 + j * P, lo)
                                            if rj >= hi:
                                                continue
                                            sc = scp.tile([P, 512], F32,
                                                          tag="sc")
                                            nc.tensor.matmul(
                                                sc[:, 0:hi - rj],
                                                lhsT=kt_sb[ho:ho + HS, hp,
                                                           k * P:
                                                           (k + 1) * P],
                                                rhs=qt_sb[ho:ho + HS, hp,
                                                          rj:hi],
                                                start=True, stop=True)
                                            nc.scalar.activation(
                                                out=ex2[:, j,
                                                        rj - lo:TH],
                                                in_=sc[:, 0:hi - rj],
                                                func=mybir.
                                                ActivationFunctionType.Exp)
                                            tb = (k % NTT) * P
                                            if lo <= tb < hi:
                                                nc.vector.tensor_mul(
                                                    out=ex2[:, j, tb - lo:
                                                            tb - lo + P],
                                                    in0=ex2[:, j, tb - lo:
                                                            tb - lo + P],
                                                    in1=msk_sb[:, k, :])
                                        if lo <= t0 < hi:
                                            # slot-1 gap ahead of its suffix
                                            nc.vector.memset(
                                                ex2[:, 1, t0 - lo:
                                                    t0 - lo + P], 0.0)
                                        # AV accumulation, split by stop runs
                                        runs = []
                                        for b0 in b256:
                                            if b0 < r0:
                                                continue
                                            stop = (lastw[b0] == m)
                                            if runs and runs[-1][2] == stop:
                                                runs[-1][1] = min(b0 + 256,
                                                                  hi)
                                            else:
                                                runs.append(
                                                    [b0, min(b0 + 256, hi),
                                                     stop])
                                        for (rb0, rb1, stop) in runs:
                                            nc.tensor.matmul(
                                                av[:, rb0 - lo:rb1 - lo],
                                                lhsT=vt_sb[:, k0:k0 + 2, h,
                                                           0:HS + 1],
                                                rhs=ex2[:, :, rb0 - lo:
                                                        rb1 - lo],
                                                start=(m == pairs[0]),
                                                stop=stop,
                                                perf_mode=mybir.
                                                MatmulPerfMode.DoubleRow)
                                    # normalize by the ones-row denominator
                                    rcp = dpool.tile([1, TH], F32, tag="rcp")
                                    nc.vector.reciprocal(
                                        out=rcp, in_=av[HS:HS + 1, :])
                                    rb = dpool.tile([HS, TH], F32, tag="rb")
                                    nc.gpsimd.partition_broadcast(rb, rcp)
                                    if h % 2 == 0:
                                        nc.vector.tensor_mul(
                                            out=att_sb[0:HS, hp, lo:hi],
                                            in0=av[0:HS, :], in1=rb)
                                    else:
                                        att_o = dpool.tile([HS, TH], BF16,
                                                           tag="atto")
                                        nc.vector.tensor_mul(
                                            out=att_o, in0=av[0:HS, :],
                                            in1=rb)
                                        nc.sync.dma_start(
                                            out=att_sb[HS:P, hp, lo:hi],
                                            in_=att_o)

                                def outproj_chunk(tt):
                                    xr_t = xrin.tile([P, D], BF16, tag="xr")
                                    nc.sync.dma_start(
                                        out=xr_t,
                                        in_=xr_ext[tt * P:(tt + 1) * P, :])
                                    po = opp.tile([P, D], F32, tag="po")
                                    for hp in range(HP):
                                        for (f0, f1) in segs(0, D):
                                            nc.tensor.matmul(
                                                po[:, f0:f1],
                                                lhsT=att_sb[:, hp, tt * P:
                                                            (tt + 1) * P],
                                                rhs=wo_sb[:, hp, f0:f1],
                                                start=(hp == 0),
                                                stop=(hp == HP - 1))
                                    nc.vector.tensor_add(
                                        out=x2_sb[:, tt, :], in0=po,
                                        in1=xr_t)

                                def ln2_chunk(tt):
                                    h2_bf = ln_pool.tile([P, D], BF16,
                                                         tag="h2bf")
                                    layernorm(x2_sb[:, tt, :], g2_sb, h2_bf)
                                    for dc in range(DC):
                                        pt = tr2.tile([P, P], BF16,
                                                      tag="pt2")
                                        nc.tensor.transpose(
                                            pt, h2_bf[:, dc * P:(dc + 1) * P],
                                            ident)
                                        if dc % 2 == 0:
                                            nc.scalar.copy(
                                                out=h2t_sb[:, dc, tt * P:
                                                           (tt + 1) * P],
                                                in_=pt)
                                        else:
                                            nc.vector.tensor_copy(
                                                out=h2t_sb[:, dc, tt * P:
                                                           (tt + 1) * P],
                                                in_=pt)

                                def mlp1_hc(hc, half, act_t):
                                    lo = half * TH
                                    w1t = w1in.tile([P, DC, P], BF16,
                                                    tag="w1")
                                    nc.sync.dma_start(out=w1t,
                                                      in_=w1view[:, :, hc, :])
                                    pm = m1p.tile([P, TH], F32, tag="pm")
                                    for dc in range(DC):
                                        nc.tensor.matmul(
                                            pm,
                                            lhsT=w1t[:, dc, :],
                                            rhs=h2t_sb[:, dc, lo:lo + TH],
                                            start=(dc == 0),
                                            stop=(dc == DC - 1))
                                    # relu(pm + b1) on DVE (keeps ACT free)
                                    nc.vector.scalar_tensor_tensor(
                                        out=act_t[:, hc, :], in0=pm,
                                        scalar=b1t_sb[:, hc:hc + 1],
                                        op0=mybir.AluOpType.add,
                                        op1=mybir.AluOpType.max,
                                        in1=zero_t[:, 0:TH])

                                # ---- emission schedule ----
                                kva = kv_units(0)     # blocks for half A
                                for u in kva:
                                    u()
                                kvb = kv_units(1)     # half B, as PE filler
                                ib = 0

                                def drain_kvb(frac):
                                    nonlocal ib
                                    tgt = int(len(kvb) * frac + 0.999)
                                    while ib < tgt:
                                        kvb[ib]()
                                        ib += 1

                                for h in range(H):
                                    attn_head(h, 0)
                                    drain_kvb((h + 1) / H)

                                # half B attention with out-proj/LN2/MLP1 of
                                # half A as PE filler under the exp window
                                fillers = []
                                for tt in range(NTT2):
                                    fillers.append(
                                        lambda tt=tt: outproj_chunk(tt))
                                    fillers.append(
                                        lambda tt=tt: ln2_chunk(tt))
                                for hc in range(HC):
                                    fillers.append(
                                        lambda hc=hc: mlp1_hc(hc, 0, act_a))
                                fi = 0
                                for h in range(H):
                                    attn_head(h, 1)
                                    tgt = int(len(fillers) * (h + 1) / H
                                              + 0.999)
                                    while fi < tgt:
                                        fillers[fi]()
                                        fi += 1

                                # ---- half B tail: out-proj, LN2, MLP1 ----
                                with tc.tile_pool(name="actb", bufs=1,
                                                  side="right") as actp_b:
                                    act_b = actp_b.tile([P, HC, TH], BF16)
                                    for tt in range(NTT2, NTT):
                                        outproj_chunk(tt)
                                        ln2_chunk(tt)
                                    for hc in range(HC):
                                        mlp1_hc(hc, 1, act_b)

                                    # ======== MLP2 + residual -> out ======
                                    w2view = w2_ext[:].rearrange(
                                        "(hc p) n -> p hc n", p=P)
                                    GRP = 4 if NTT % 4 == 0 else 2
                                    with tc.tile_pool(name="w2in",
                                                      bufs=4) as w2in, \
                                         tc.tile_pool(name="opool",
                                                      bufs=3) as opool, \
                                         tc.tile_pool(name="m2_psum", bufs=1,
                                                      space="PSUM") as m2p:
                                        for g in range(NTT // GRP):
                                            psums = [
                                                m2p.tile([P, D], F32,
                                                         name=f"m2ps{_t}",
                                                         tag=f"m2ps{_t}")
                                                for _t in range(GRP)]
                                            for hc in range(HC):
                                                w2t = w2in.tile([P, D], BF16,
                                                                tag="w2")
                                                nc.sync.dma_start(
                                                    out=w2t,
                                                    in_=w2view[:, hc, :])
                                                for ti in range(GRP):
                                                    tt = g * GRP + ti
                                                    act_t = (act_a
                                                             if tt < NTT2
                                                             else act_b)
                                                    ac = (tt % NTT2) * P
                                                    for (f0, f1) in segs(
                                                            0, D):
                                                        nc.tensor.matmul(
                                                            psums[ti][:,
                                                                      f0:f1],
                                                            lhsT=act_t[
                                                                :, hc,
                                                                ac:ac + P],
                                                            rhs=w2t[:,
                                                                    f0:f1],
                                                            start=(hc == 0),
                                                            stop=(hc ==
                                                                  HC - 1))
                                            for ti in range(GRP):
                                                tt = g * GRP + ti
                                                o_sb = opool.tile(
                                                    [P, D], F32, tag="o")
                                                nc.vector.tensor_add(
                                                    out=o_sb,
                                                    in0=psums[ti],
                                                    in1=x2_sb[:, tt, :])
                                                nc.vector.tensor_add(
                                                    out=o_sb, in0=o_sb,
                                                    in1=b2_sb)
                                                nc.sync.dma_start(
                                                    out=out_ext[
                                                        tt * P:(tt + 1) * P,
                                                        :],
                                                    in_=o_sb)

                        nc.sync.dma_start(out=g2_sb, in_=bcast_row(g2_ext))
                        nc.sync.dma_start(out=b2_sb, in_=bcast_row(b2_ext))
                        nc.sync.dma_start(out=b1t_sb, in_=b1t_ext[:])

    nc.finalize()
    return nc


def shard_inputs(cfg: Cfg, inputs):
    """Full inputs (reference layout) -> per-core in_maps in kernel layout."""
    B, T, D, DH, HS, NC = cfg.B, cfg.T, cfg.D, cfg.DH, cfg.HS, cfg.NC
    NCH, NTT, NKB = cfg.NCH, cfg.NTT, cfg.NKB
    f32 = np.float32
    x = np.asarray(inputs["x"], f32)                     # [B, T, D]
    Wq = np.asarray(inputs["Wq"], f32)                   # (H, D, HS)
    Wk = np.asarray(inputs["Wk"], f32)
    Wv = np.asarray(inputs["Wv"], f32)
    wq = np.ascontiguousarray(
        Wq.transpose(1, 0, 2).reshape(D, D) * (HS ** -0.5)).astype(NP_F8)
    wk = np.ascontiguousarray(
        Wk.transpose(1, 0, 2).reshape(D, D)).astype(NP_F8)
    wv = np.ascontiguousarray(
        Wv.transpose(1, 0, 2).reshape(D, D)).astype(NP_F8)
    Wo = np.ascontiguousarray(np.asarray(inputs["Wo"], f32)).astype(NP_BF16)
    W1 = np.ascontiguousarray(np.asarray(inputs["W1"], f32)).astype(NP_BF16)
    W2 = np.ascontiguousarray(np.asarray(inputs["W2"], f32)).astype(NP_BF16)
    row = lambda v: np.asarray(v, f32).reshape(1, D)
    g1, g2 = row(inputs["g1"]), row(inputs["g2"])
    b2 = row(inputs["b2"])
    xr = x + np.asarray(inputs["bo"], f32).reshape(1, D)
    b1t = np.ascontiguousarray(
        np.asarray(inputs["b1"], f32).reshape(cfg.HC, P).T)

    tri = (np.arange(P)[:, None] <= np.arange(P)[None, :])  # keys u <= qry t
    ev, od = own_chunks(NCH, 0), own_chunks(NCH, 1)
    blocks = ev + od        # key block -> absolute chunk (slice 0 then 1)

    in_maps = []
    for c in range(NC):
        b, par = c // 2, c % 2
        own = ev if par == 0 else od
        sel = np.concatenate(
            [np.arange(ch * P, (ch + 1) * P) for ch in own])
        msk = np.zeros((P, NKB, P), f32)
        for k in range(NKB):
            key_ch = blocks[k]             # absolute chunk of key block k
            qry_ch = own[k % NTT]          # own chunk at padded position
            if qry_ch == key_ch:
                msk[:, k, :] = tri
            elif qry_ch > key_ch:
                msk[:, k, :] = 1.0
            # else: stays zero (non-causal padding block)
        in_maps.append({
            "x": np.ascontiguousarray(x[b][sel]),
            "xr": np.ascontiguousarray(xr[b][sel]).astype(NP_BF16),
            "wq": wq, "wk": wk, "wv": wv,
            "wo": Wo, "w1": W1, "w2": W2,
            "g1": g1, "g2": g2,
            "b2": b2, "b1t": b1t,
            "msk": np.ascontiguousarray(msk).astype(NP_BF16),
        })
    return in_maps


_cache = {}


def _get_nc(cfg: Cfg, reps: int = 1):
    key = (cfg.B, cfg.T, cfg.D, cfg.DH, reps)
    if key not in _cache:
        _cache[key] = build_nc(cfg, reps)
    return _cache[key]


def assemble(cfg: Cfg, shards) -> np.ndarray:
    """Per-core [TSH, D] packed outputs -> [B, T, D]."""
    out = np.empty((cfg.B, cfg.T, cfg.D), np.float32)
    for c in range(cfg.NC):
        b, par = c // 2, c % 2
        own = own_chunks(cfg.NCH, par)
        sh = np.asarray(shards[c])
        for j, ch in enumerate(own):
            out[b, ch * P:(ch + 1) * P, :] = sh[j * P:(j + 1) * P]
    return out


def kernel(**inputs) -> np.ndarray:
    cfg = FULL
    nc = _get_nc(cfg)
    in_maps = shard_inputs(cfg, inputs)
    res = run_bass_kernel_spmd(nc, in_maps, core_ids=list(range(cfg.NC)))
    return assemble(cfg, [res.results[c]["out"] for c in range(cfg.NC)])
